# revision 19
# baseline (speedup 1.0000x reference)
"""MinLSTM layer on 8 Trainium2 NeuronCores.

Math (equivalent to the log-space reference, done in linear space):
    f_pre = x @ W_f.T + b_f ; i_pre = x @ W_i.T + b_i ; h_pre = x @ W_h.T + b_h
    sf = sigmoid(f_pre) ; si = sigmoid(i_pre)
    f = sf / (sf + si)                       # normalized forget gate
    i = 1 - f                                # = si / (sf + si)
    g = max(sigmoid(h_pre), h_pre + 0.5)     # == exp(log_g), exactly
    h_t = f_t * h_{t-1} + i_t * g_t,  h_0 = 1
The gates satisfy f in (0,1), g > 0, so h stays in a tame range and the
recurrence is numerically stable in fp32 (max rel err vs the fp32 log-space
reference ~1e-3 with fp16 matmul operands; fp32 PSUM accumulation).

Sharding: 8 cores = batch(4) x hidden-halves(2). Core c handles batch b=c//2,
hidden slice [(c%2)*512, (c%2+1)*512). No cross-core communication; the scan
runs along T inside each core via the DVE TensorTensorScan instruction
(state = f*state - mv per step, mv = (f-1)*g = -i*g).

Device layout: gates computed as [h_part, t_free] via out = W_sliceT.T @ xT;
host pre-transposes/packs x and W (numpy) and re-transposes the [512, 4096]
per-core output back to [T, Dh]. Matmuls run in 512-wide t-chunks (one PSUM
bank); elementwise+scan run in up-to-1024-wide super-chunks, with the scan
carry passed as the previous chunk's last column.

Performance notes (see git/comment history for the trace evidence):
- fp16 matmul operands: 1 cycle/row PE stream (like fp32r>=256) but the
  per-matmul self-loading LDWEIGHTS drops ~188ns -> ~97ns, which hides under
  the 213ns stream window: back-to-back matmuls run at ~216ns/512-row matmul,
  the PE stream floor. (fp32r forces a 4-byte weight reload per matmul that
  cannot be hidden: +16ns each.)
- k-pair packing: x and W are packed on host so each DMA moves [128, 1024]
  fp16 = 2KB per partition line. 1KB lines (naive fp16 tiles) measurably cap
  the HBM stream at ~150-250 GB/s; 2KB lines restore ~330 GB/s, which the
  early J0 weight+x demand needs.
- J0 is narrow (512 t) and gate-major, k-outer, in DMA priority order
  (x0[kk], W_f[kk]) -> bias -> W_i -> W_h, so the PE chases the ramping HBM
  stream with minimal stalls while all three weight matrices arrive.
- 7 full-width warmup matmuls on a zeroed scratch tile fill the engine
  preamble -> first-data window and ramp the PE HAM clock gate to 2.4 GHz.
- The last (chunk, h-tile) unit is restructured: f/i gates run first and
  their normalization overlaps the h-gate matmuls; the h gate then drains in
  256-wide ACT -> g -> mv -> scan -> store grains to shorten the serial tail.
"""

import sys

for _p in ("/opt/trn_rl_repo",):
    if _p not in sys.path:
        sys.path.append(_p)

import numpy as np

import concourse.bass as bass
import concourse.tile as tile
from concourse import bacc, mybir
from concourse.bass_utils import run_bass_kernel_spmd

B, T, DIN, DH = 4, 4096, 1024, 1024
N_CORES = 8
HSH = DH // 2          # 512 hidden channels per core
P = 128                # partitions
KT = DIN // P          # 8 contraction tiles
KK = KT // 2           # 4 packed k-pairs (2KB DMA lines)
NT = 512               # matmul t-chunk (free dim, one PSUM bank)
NC = T // NT           # 8 t-chunks
IT = HSH // P          # 4 h-tiles per core
# elementwise/scan super-chunks (start, length). J0 is narrow (512) so the
# weight DMAs interleave with only one chunk of x while the HBM stream ramps.
CHUNKS = [(0, 512), (512, 1024), (1536, 1024), (2560, 1024), (3584, 512)]

MM_DT = mybir.dt.float16
MM_NP = np.float16

_COMPILED = None


def _build():
    AF = mybir.ActivationFunctionType
    OP = mybir.AluOpType
    f32 = mybir.dt.float32

    nc = bacc.Bacc("TRN2", target_bir_lowering=False, debug=False)

    # x packed as [p, chunk, kk, (j t)]: j = k-pair half, t in 0..511
    xT = nc.dram_tensor("xT", [P, T * KT], MM_DT, kind="ExternalInput").ap()
    x_v = xT.rearrange("p (c kk tt) -> p c kk tt", c=NC, kk=KK)
    # W packed as [p, kk, (j h)]: 2KB per partition line
    wd = {g: nc.dram_tensor(f"w{g}", [P, KK * 2 * HSH], MM_DT,
                            kind="ExternalInput").ap()
          for g in ("f", "i", "h")}
    w_v = {g: w.rearrange("p (kk z) -> p kk z", kk=KK) for g, w in wd.items()}
    # packed per-partition scalars: [b_f | b_i | b_h | b_h+0.5], each (128, IT)
    biases = nc.dram_tensor("biases", [P, 4 * IT], f32, kind="ExternalInput").ap()
    out = nc.dram_tensor("out", [HSH, T], f32, kind="ExternalOutput").ap()

    with tile.TileContext(nc) as tc:
        with (
            tc.tile_pool(name="wpool", bufs=1) as wpool,
            tc.tile_pool(name="bpool", bufs=1) as bpool,
            tc.tile_pool(name="xpool", bufs=24) as xpool,
            tc.tile_pool(name="psum", bufs=8, space="PSUM") as pspool,
            tc.tile_pool(name="work", bufs=4) as work,
            tc.tile_pool(name="hpool", bufs=6) as hpool,
        ):
            bias_t = bpool.tile([P, 4 * IT], f32, tag="bias")

            # per-kk weight tiles ([128, 1024] = both k halves), resident
            wt = {g: [wpool.tile([P, 2 * HSH], MM_DT, tag=f"w{g}{kk}",
                                 name=f"w{g}{kk}_t")
                      for kk in range(KK)] for g in ("f", "i", "h")}

            def wsl(g, k, i):
                kk, j = divmod(k, 2)
                c0 = j * HSH + i * P
                return wt[g][kk][:, c0:c0 + P]

            def dma_w(g):
                for kk in range(KK):
                    nc.sync.dma_start(out=wt[g][kk][:], in_=w_v[g][:, kk, :])

            def x_ktiles(c):
                """KK [P, 1024] pack tiles (both k halves) of t-chunk c."""
                xs = []
                for kk in range(KK):
                    xk = xpool.tile([P, 2 * NT], MM_DT, tag="xk", name="xk_t")
                    nc.sync.dma_start(out=xk[:], in_=x_v[:, c, kk, :])
                    xs.append(xk)
                return xs

            def xsl(xs, k):
                kk, j = divmod(k, 2)
                return xs[kk][:, j * NT:(j + 1) * NT]

            def bias_ap(kind, i):
                return bias_t[:, kind * IT + i:kind * IT + i + 1]

            def chain(i, sf, si, sg, gt, J, t0, ne, grain=None, pool_tt=False):
                """Normalize gates, build -i*g, scan, and store chunk.

                pool_tt moves the add/mul to the otherwise-idle GPSIMD
                engine: used for the final chunk's non-drain units so the
                in-order DVE queue is drained before the last unit's
                post-matmul chain (which bounds the kernel tail).
                """
                tt = nc.gpsimd if pool_tt else nc.vector
                grain = grain or ne
                for c0 in range(0, ne, grain):
                    cs = slice(c0, c0 + grain)
                    tt.tensor_add(si[:, cs], sf[:, cs], si[:, cs])
                    r = work.tile([P, grain], f32, tag="sg", name="r_t")
                    nc.vector.reciprocal_approx_fast(out=r[:], in_=si[:, cs])
                    tt.tensor_mul(sf[:, cs], sf[:, cs], r[:])             # f
                    nc.vector.scalar_tensor_tensor(                # mv=(f-1)*g
                        gt[:, cs], sf[:, cs], 1.0, gt[:, cs],
                        op0=OP.subtract, op1=OP.mult)
                    hc = hpool.tile([P, grain], f32, tag="h", name=f"h{i}_t")
                    init = 1.0 if J == 0 and c0 == 0 else hprev[i][:, -1:]
                    nc.vector.tensor_tensor_scan(
                        hc[:], sf[:, cs], gt[:, cs], init,
                        op0=OP.mult, op1=OP.subtract)
                    hprev[i] = hc
                    nc.sync.dma_start(
                        out=out[i * P:(i + 1) * P, t0 + c0:t0 + c0 + grain],
                        in_=hc[:])

            hprev = [None] * IT

            # Warmup matmuls on a zeroed scratch tile: fill the engine
            # preamble -> first-data window at full array duty so the PE HAM
            # clock gate ramps to 2.4 GHz before real matmuls start.
            scratch = bpool.tile([P, NT], MM_DT, tag="scratch")
            nc.vector.memset(scratch[:].bitcast(mybir.dt.uint32), 0)
            pswarm = pspool.tile([P, NT], f32, tag="ps", name="pswarm_t")
            for _ in range(7):
                nc.tensor.matmul(pswarm[:], lhsT=scratch[:, :P], rhs=scratch[:],
                                 start=True, stop=True)

            # ---- J0: narrow (512), gate-major, k-outer; PE chases the DMA
            # stream while W_f/W_i/W_h arrive ----
            t0, ne = CHUNKS[0]
            xc0 = []
            for kk in range(KK):
                xk = xpool.tile([P, 2 * NT], MM_DT, tag="xk", name="xk_t")
                nc.sync.dma_start(out=xk[:], in_=x_v[:, 0, kk, :])
                nc.sync.dma_start(out=wt["f"][kk][:], in_=w_v["f"][:, kk, :])
                if kk == 0:
                    nc.sync.dma_start(out=bias_t[:], in_=biases[:])
                xc0.append(xk)
            dma_w("i")
            dma_w("h")

            sf = [work.tile([P, ne], f32, tag="sf", name="sf_t") for _ in range(IT)]
            si = [work.tile([P, ne], f32, tag="si", name="si_t") for _ in range(IT)]
            sg = [work.tile([P, ne], f32, tag="sg", name="sg_t") for _ in range(IT)]
            gt = [work.tile([P, ne], f32, tag="gt", name="gt_t") for _ in range(IT)]
            for gate, dsts, bk in (("f", sf, 0), ("i", si, 1), ("h", sg, 2)):
                psts = [pspool.tile([P, NT], f32, tag="ps", name="ps_t")
                        for _ in range(IT)]
                for k in range(KT):
                    for i, pst in enumerate(psts):
                        nc.tensor.matmul(
                            pst[:], lhsT=wsl(gate, k, i), rhs=xsl(xc0, k),
                            start=(k == 0), stop=(k == KT - 1))
                for i in range(IT):
                    nc.scalar.activation(dsts[i][:], psts[i][:], AF.Sigmoid,
                                         bias=bias_ap(bk, i), scale=1.0)
                    if gate == "h":
                        nc.vector.scalar_tensor_tensor(
                            gt[i][:], psts[i][:], bias_ap(3, i),
                            sg[i][:], op0=OP.add, op1=OP.max)
            for i in range(IT):
                chain(i, sf[i], si[i], sg[i], gt[i], 0, t0, ne)

            # ---- J1+: h-tile-major units ----
            for J, (t0, ne) in enumerate(CHUNKS[1:], start=1):
                nhalf = ne // NT
                xcs = [x_ktiles(t0 // NT + h) for h in range(nhalf)]
                last_J = J == len(CHUNKS) - 1
                for i in range(IT):
                    last_unit = last_J and i == IT - 1
                    sf = work.tile([P, ne], f32, tag="sf", name="sf_t")
                    si = work.tile([P, ne], f32, tag="si", name="si_t")
                    sg = work.tile([P, ne], f32, tag="sg", name="sg_t")
                    gt = work.tile([P, ne], f32, tag="gt", name="gt_t")
                    if not last_unit:
                        for half in range(nhalf):
                            esl = slice(half * NT, (half + 1) * NT)
                            for gate, dst, bk in (("f", sf, 0), ("i", si, 1),
                                                  ("h", sg, 2)):
                                pst = pspool.tile([P, NT], f32, tag="ps",
                                                  name="ps_t")
                                for k in range(KT):
                                    nc.tensor.matmul(
                                        pst[:], lhsT=wsl(gate, k, i),
                                        rhs=xsl(xcs[half], k),
                                        start=(k == 0), stop=(k == KT - 1))
                                nc.scalar.activation(dst[:, esl], pst[:],
                                                     AF.Sigmoid,
                                                     bias=bias_ap(bk, i),
                                                     scale=1.0)
                                if gate == "h":
                                    nc.vector.scalar_tensor_tensor(
                                        gt[:, esl], pst[:], bias_ap(3, i),
                                        sg[:, esl], op0=OP.add, op1=OP.max)
                        chain(i, sf, si, sg, gt, J, t0, ne, pool_tt=last_J)
                        continue

                    # Last unit: f/i gates first (their normalization runs on
                    # the DVE under the h-gate matmuls), then the h gate
                    # drains in 256-wide ACT -> g -> mv -> scan -> store
                    # grains to shorten the serial post-matmul tail.
                    GR = 256
                    for gate, dst, bk in (("f", sf, 0), ("i", si, 1)):
                        for half in range(nhalf):
                            esl = slice(half * NT, (half + 1) * NT)
                            pst = pspool.tile([P, NT], f32, tag="ps",
                                              name="ps_t")
                            for k in range(KT):
                                nc.tensor.matmul(
                                    pst[:], lhsT=wsl(gate, k, i),
                                    rhs=xsl(xcs[half], k),
                                    start=(k == 0), stop=(k == KT - 1))
                            nc.scalar.activation(dst[:, esl], pst[:],
                                                 AF.Sigmoid,
                                                 bias=bias_ap(bk, i), scale=1.0)
                    for c0 in range(0, ne, GR):
                        cs = slice(c0, c0 + GR)
                        nc.vector.tensor_add(si[:, cs], sf[:, cs], si[:, cs])
                        r = work.tile([P, GR], f32, tag="sg", name="r_t")
                        nc.vector.reciprocal_approx_fast(out=r[:], in_=si[:, cs])
                        nc.vector.tensor_mul(sf[:, cs], sf[:, cs], r[:])
                    for half in range(nhalf):
                        pst = pspool.tile([P, NT], f32, tag="ps", name="ps_t")
                        for k in range(KT):
                            nc.tensor.matmul(
                                pst[:], lhsT=wsl("h", k, i),
                                rhs=xsl(xcs[half], k),
                                start=(k == 0), stop=(k == KT - 1))
                        for c0 in range(0, NT, GR):
                            e0 = half * NT + c0
                            esl = slice(e0, e0 + GR)
                            psl = slice(c0, c0 + GR)
                            nc.scalar.activation(sg[:, esl], pst[:, psl],
                                                 AF.Sigmoid,
                                                 bias=bias_ap(2, i), scale=1.0)
                            nc.vector.scalar_tensor_tensor(
                                gt[:, esl], pst[:, psl], bias_ap(3, i),
                                sg[:, esl], op0=OP.add, op1=OP.max)
                            nc.vector.scalar_tensor_tensor(
                                gt[:, esl], sf[:, esl], 1.0, gt[:, esl],
                                op0=OP.subtract, op1=OP.mult)
                            hc = hpool.tile([P, GR], f32, tag="h",
                                            name=f"h{i}_t")
                            nc.vector.tensor_tensor_scan(
                                hc[:], sf[:, esl], gt[:, esl],
                                hprev[i][:, -1:],
                                op0=OP.mult, op1=OP.subtract)
                            hprev[i] = hc
                            nc.sync.dma_start(
                                out=out[i * P:(i + 1) * P,
                                        t0 + e0:t0 + e0 + GR],
                                in_=hc[:])

    nc.compile()
    return nc


def _in_maps(x, W_f, b_f, W_i, b_i, W_h, b_h):
    x = np.asarray(x, MM_NP)
    wT = {g: np.asarray(w, np.float32).T.astype(MM_NP)
          for g, w in (("f", W_f), ("i", W_i), ("h", W_h))}
    bs = {g: np.asarray(b, np.float32) for g, b in (("f", b_f), ("i", b_i), ("h", b_h))}

    maps = []
    for c in range(N_CORES):
        b, hh = divmod(c, 2)
        hsl = slice(hh * HSH, (hh + 1) * HSH)
        bias_pack = np.concatenate([
            bs["f"][hsl].reshape(IT, P).T,
            bs["i"][hsl].reshape(IT, P).T,
            bs["h"][hsl].reshape(IT, P).T,
            (bs["h"][hsl] + 0.5).reshape(IT, P).T,
        ], axis=1)
        # x pack: [p, c, kk, j, t] = xT[(2kk+j)*P + p, c*NT + t]
        xb = np.ascontiguousarray(x[b].T)                    # (DIN, T)
        xp = xb.reshape(KK, 2, P, NC, NT).transpose(2, 3, 0, 1, 4)
        # W pack: [p, kk, j, h] = W^T[(2kk+j)*P + p, h]
        wp = {g: wT[g][:, hsl].reshape(KK, 2, P, HSH).transpose(2, 0, 1, 3)
              for g in ("f", "i", "h")}
        maps.append({
            "xT": np.ascontiguousarray(xp.reshape(P, T * KT)),
            "wf": np.ascontiguousarray(wp["f"].reshape(P, KK * 2 * HSH)),
            "wi": np.ascontiguousarray(wp["i"].reshape(P, KK * 2 * HSH)),
            "wh": np.ascontiguousarray(wp["h"].reshape(P, KK * 2 * HSH)),
            "biases": np.ascontiguousarray(bias_pack, dtype=np.float32),
        })
    return maps


def kernel(x, W_f, b_f, W_i, b_i, W_h, b_h):
    global _COMPILED
    if _COMPILED is None:
        _COMPILED = _build()
    nc = _COMPILED

    res = run_bass_kernel_spmd(
        nc, _in_maps(x, W_f, b_f, W_i, b_i, W_h, b_h), list(range(N_CORES)))

    full = np.empty((B, T, DH), np.float32)
    for c in range(N_CORES):
        b, hh = divmod(c, 2)
        full[b, :, hh * HSH:(hh + 1) * HSH] = res.results[c]["out"].T
    return full


# revision 20
# speedup vs baseline: 1.0071x; 1.0071x over previous
"""MinLSTM layer on 8 Trainium2 NeuronCores.

Math (equivalent to the log-space reference, done in linear space):
    f_pre = x @ W_f.T + b_f ; i_pre = x @ W_i.T + b_i ; h_pre = x @ W_h.T + b_h
    sf = sigmoid(f_pre) ; si = sigmoid(i_pre)
    f = sf / (sf + si)                       # normalized forget gate
    i = 1 - f                                # = si / (sf + si)
    g = max(sigmoid(h_pre), h_pre + 0.5)     # == exp(log_g), exactly
    h_t = f_t * h_{t-1} + i_t * g_t,  h_0 = 1
The gates satisfy f in (0,1), g > 0, so h stays in a tame range and the
recurrence is numerically stable in fp32 (max rel err vs the fp32 log-space
reference ~1e-3 with fp16 matmul operands; fp32 PSUM accumulation).

Sharding: 8 cores = batch(4) x hidden-halves(2). Core c handles batch b=c//2,
hidden slice [(c%2)*512, (c%2+1)*512). No cross-core communication; the scan
runs along T inside each core via the DVE TensorTensorScan instruction
(state = f*state - mv per step, mv = (f-1)*g = -i*g).

Device layout: gates computed as [h_part, t_free] via out = W_sliceT.T @ xT;
host pre-transposes/packs x and W (numpy) and re-transposes the [512, 4096]
per-core output back to [T, Dh]. Matmuls run in 512-wide t-chunks (one PSUM
bank); elementwise+scan run in up-to-1024-wide super-chunks, with the scan
carry passed as the previous chunk's last column.

Performance notes (see git/comment history for the trace evidence):
- fp16 matmul operands: 1 cycle/row PE stream (like fp32r>=256) but the
  per-matmul self-loading LDWEIGHTS drops ~188ns -> ~97ns, which hides under
  the 213ns stream window: back-to-back matmuls run at ~216ns/512-row matmul,
  the PE stream floor. (fp32r forces a 4-byte weight reload per matmul that
  cannot be hidden: +16ns each.)
- k-pair packing: x and W are packed on host so each DMA moves [128, 1024]
  fp16 = 2KB per partition line. 1KB lines (naive fp16 tiles) measurably cap
  the HBM stream at ~150-250 GB/s; 2KB lines restore ~330 GB/s, which the
  early J0 weight+x demand needs.
- J0 is narrow (512 t) and gate-major, k-outer, in DMA priority order
  (x0[kk], W_f[kk]) -> bias -> W_i -> W_h, so the PE chases the ramping HBM
  stream with minimal stalls while all three weight matrices arrive.
- 7 full-width warmup matmuls on a zeroed scratch tile fill the engine
  preamble -> first-data window and ramp the PE HAM clock gate to 2.4 GHz.
- The last (chunk, h-tile) unit is restructured: f/i gates run first and
  their normalization overlaps the h-gate matmuls; the h gate then drains in
  256-wide ACT -> g -> mv -> scan -> store grains to shorten the serial tail.
"""

import sys

for _p in ("/opt/trn_rl_repo",):
    if _p not in sys.path:
        sys.path.append(_p)

import numpy as np

import concourse.bass as bass
import concourse.tile as tile
from concourse import bacc, mybir
from concourse.bass_utils import run_bass_kernel_spmd

B, T, DIN, DH = 4, 4096, 1024, 1024
N_CORES = 8
HSH = DH // 2          # 512 hidden channels per core
P = 128                # partitions
KT = DIN // P          # 8 contraction tiles
KK = KT // 2           # 4 packed k-pairs (2KB DMA lines)
NT = 512               # matmul t-chunk (free dim, one PSUM bank)
NC = T // NT           # 8 t-chunks
IT = HSH // P          # 4 h-tiles per core
# elementwise/scan super-chunks (start, length). J0 is narrow (512) so the
# weight DMAs interleave with only one chunk of x while the HBM stream ramps.
CHUNKS = [(0, 512), (512, 1024), (1536, 1024), (2560, 1024), (3584, 512)]

MM_DT = mybir.dt.float16
MM_NP = np.float16

_COMPILED = None


def _build():
    AF = mybir.ActivationFunctionType
    OP = mybir.AluOpType
    f32 = mybir.dt.float32

    nc = bacc.Bacc("TRN2", target_bir_lowering=False, debug=False)

    # x packed as [p, chunk, kk, (j t)]: j = k-pair half, t in 0..511
    xT = nc.dram_tensor("xT", [P, T * KT], MM_DT, kind="ExternalInput").ap()
    x_v = xT.rearrange("p (c kk tt) -> p c kk tt", c=NC, kk=KK)
    # W packed as [p, kk, (j h)]: 2KB per partition line
    wd = {g: nc.dram_tensor(f"w{g}", [P, KK * 2 * HSH], MM_DT,
                            kind="ExternalInput").ap()
          for g in ("f", "i", "h")}
    w_v = {g: w.rearrange("p (kk z) -> p kk z", kk=KK) for g, w in wd.items()}
    # packed per-partition scalars: [b_f | b_i | b_h | b_h+0.5], each (128, IT)
    biases = nc.dram_tensor("biases", [P, 4 * IT], f32, kind="ExternalInput").ap()
    out = nc.dram_tensor("out", [HSH, T], f32, kind="ExternalOutput").ap()

    with tile.TileContext(nc) as tc:
        with (
            tc.tile_pool(name="wpool", bufs=1) as wpool,
            tc.tile_pool(name="bpool", bufs=1) as bpool,
            tc.tile_pool(name="xpool", bufs=24) as xpool,
            tc.tile_pool(name="psum", bufs=8, space="PSUM") as pspool,
            tc.tile_pool(name="work", bufs=4) as work,
            tc.tile_pool(name="hpool", bufs=6) as hpool,
        ):
            bias_t = bpool.tile([P, 4 * IT], f32, tag="bias")

            # per-kk weight tiles ([128, 1024] = both k halves), resident
            wt = {g: [wpool.tile([P, 2 * HSH], MM_DT, tag=f"w{g}{kk}",
                                 name=f"w{g}{kk}_t")
                      for kk in range(KK)] for g in ("f", "i", "h")}

            def wsl(g, k, i):
                kk, j = divmod(k, 2)
                c0 = j * HSH + i * P
                return wt[g][kk][:, c0:c0 + P]

            def dma_w(g):
                for kk in range(KK):
                    nc.sync.dma_start(out=wt[g][kk][:], in_=w_v[g][:, kk, :])

            def x_ktiles(c):
                """KK [P, 1024] pack tiles (both k halves) of t-chunk c."""
                xs = []
                for kk in range(KK):
                    xk = xpool.tile([P, 2 * NT], MM_DT, tag="xk", name="xk_t")
                    nc.sync.dma_start(out=xk[:], in_=x_v[:, c, kk, :])
                    xs.append(xk)
                return xs

            def xsl(xs, k):
                kk, j = divmod(k, 2)
                return xs[kk][:, j * NT:(j + 1) * NT]

            def bias_ap(kind, i):
                return bias_t[:, kind * IT + i:kind * IT + i + 1]

            def chain(i, sf, si, sg, gt, J, t0, ne, grain=None, pool_tt=False):
                """Normalize gates, build -i*g, scan, and store chunk.

                pool_tt moves the add/mul to the otherwise-idle GPSIMD
                engine: used for the final chunk's non-drain units so the
                in-order DVE queue is drained before the last unit's
                post-matmul chain (which bounds the kernel tail).
                """
                tt = nc.gpsimd if pool_tt else nc.vector
                grain = grain or ne
                for c0 in range(0, ne, grain):
                    cs = slice(c0, c0 + grain)
                    tt.tensor_add(si[:, cs], sf[:, cs], si[:, cs])
                    r = work.tile([P, grain], f32, tag="sg", name="r_t")
                    nc.vector.reciprocal_approx_fast(out=r[:], in_=si[:, cs])
                    tt.tensor_mul(sf[:, cs], sf[:, cs], r[:])             # f
                    nc.vector.scalar_tensor_tensor(                # mv=(f-1)*g
                        gt[:, cs], sf[:, cs], 1.0, gt[:, cs],
                        op0=OP.subtract, op1=OP.mult)
                    hc = hpool.tile([P, grain], f32, tag="h", name=f"h{i}_t")
                    init = 1.0 if J == 0 and c0 == 0 else hprev[i][:, -1:]
                    nc.vector.tensor_tensor_scan(
                        hc[:], sf[:, cs], gt[:, cs], init,
                        op0=OP.mult, op1=OP.subtract)
                    hprev[i] = hc
                    nc.sync.dma_start(
                        out=out[i * P:(i + 1) * P, t0 + c0:t0 + c0 + grain],
                        in_=hc[:])

            hprev = [None] * IT

            # Warmup matmuls on a zeroed scratch tile: fill the engine
            # preamble -> first-data window at full array duty so the PE HAM
            # clock gate ramps to 2.4 GHz before real matmuls start.
            scratch = bpool.tile([P, NT], MM_DT, tag="scratch")
            nc.vector.memset(scratch[:].bitcast(mybir.dt.uint32), 0)
            pswarm = pspool.tile([P, NT], f32, tag="ps", name="pswarm_t")
            for _ in range(7):
                nc.tensor.matmul(pswarm[:], lhsT=scratch[:, :P], rhs=scratch[:],
                                 start=True, stop=True)

            # ---- J0: narrow (512), gate-major, k-outer; PE chases the DMA
            # stream while W_f/W_i/W_h arrive ----
            t0, ne = CHUNKS[0]
            xc0 = []
            for kk in range(KK):
                xk = xpool.tile([P, 2 * NT], MM_DT, tag="xk", name="xk_t")
                nc.sync.dma_start(out=xk[:], in_=x_v[:, 0, kk, :])
                nc.sync.dma_start(out=wt["f"][kk][:], in_=w_v["f"][:, kk, :])
                if kk == 0:
                    nc.sync.dma_start(out=bias_t[:], in_=biases[:])
                xc0.append(xk)
            dma_w("i")
            dma_w("h")

            sf = [work.tile([P, ne], f32, tag="sf", name="sf_t") for _ in range(IT)]
            si = [work.tile([P, ne], f32, tag="si", name="si_t") for _ in range(IT)]
            sg = [work.tile([P, ne], f32, tag="sg", name="sg_t") for _ in range(IT)]
            gt = [work.tile([P, ne], f32, tag="gt", name="gt_t") for _ in range(IT)]
            for gate, dsts, bk in (("f", sf, 0), ("i", si, 1), ("h", sg, 2)):
                psts = [pspool.tile([P, NT], f32, tag="ps", name="ps_t")
                        for _ in range(IT)]
                for k in range(KT):
                    for i, pst in enumerate(psts):
                        nc.tensor.matmul(
                            pst[:], lhsT=wsl(gate, k, i), rhs=xsl(xc0, k),
                            start=(k == 0), stop=(k == KT - 1))
                for i in range(IT):
                    nc.scalar.activation(dsts[i][:], psts[i][:], AF.Sigmoid,
                                         bias=bias_ap(bk, i), scale=1.0)
                    if gate == "h":
                        nc.vector.scalar_tensor_tensor(
                            gt[i][:], psts[i][:], bias_ap(3, i),
                            sg[i][:], op0=OP.add, op1=OP.max)
            for i in range(IT):
                chain(i, sf[i], si[i], sg[i], gt[i], 0, t0, ne)

            # ---- J1+: h-tile-major units ----
            for J, (t0, ne) in enumerate(CHUNKS[1:], start=1):
                nhalf = ne // NT
                xcs = [x_ktiles(t0 // NT + h) for h in range(nhalf)]
                last_J = J == len(CHUNKS) - 1
                for i in range(IT):
                    last_unit = last_J and i == IT - 1
                    sf = work.tile([P, ne], f32, tag="sf", name="sf_t")
                    si = work.tile([P, ne], f32, tag="si", name="si_t")
                    sg = work.tile([P, ne], f32, tag="sg", name="sg_t")
                    gt = work.tile([P, ne], f32, tag="gt", name="gt_t")
                    if not last_unit:
                        for half in range(nhalf):
                            esl = slice(half * NT, (half + 1) * NT)
                            for gate, dst, bk in (("f", sf, 0), ("i", si, 1),
                                                  ("h", sg, 2)):
                                pst = pspool.tile([P, NT], f32, tag="ps",
                                                  name="ps_t")
                                for k in range(KT):
                                    nc.tensor.matmul(
                                        pst[:], lhsT=wsl(gate, k, i),
                                        rhs=xsl(xcs[half], k),
                                        start=(k == 0), stop=(k == KT - 1))
                                nc.scalar.activation(dst[:, esl], pst[:],
                                                     AF.Sigmoid,
                                                     bias=bias_ap(bk, i),
                                                     scale=1.0)
                                if gate == "h":
                                    nc.vector.scalar_tensor_tensor(
                                        gt[:, esl], pst[:], bias_ap(3, i),
                                        sg[:, esl], op0=OP.add, op1=OP.max)
                        chain(i, sf, si, sg, gt, J, t0, ne)
                        continue

                    # Last unit: f/i gates first (their normalization runs on
                    # the DVE under the h-gate matmuls), then the h gate
                    # drains in 256-wide ACT -> g -> mv -> scan -> store
                    # grains to shorten the serial post-matmul tail.
                    GR = 256
                    for gate, dst, bk in (("f", sf, 0), ("i", si, 1)):
                        for half in range(nhalf):
                            esl = slice(half * NT, (half + 1) * NT)
                            pst = pspool.tile([P, NT], f32, tag="ps",
                                              name="ps_t")
                            for k in range(KT):
                                nc.tensor.matmul(
                                    pst[:], lhsT=wsl(gate, k, i),
                                    rhs=xsl(xcs[half], k),
                                    start=(k == 0), stop=(k == KT - 1))
                            nc.scalar.activation(dst[:, esl], pst[:],
                                                 AF.Sigmoid,
                                                 bias=bias_ap(bk, i), scale=1.0)
                    for c0 in range(0, ne, GR):
                        cs = slice(c0, c0 + GR)
                        nc.vector.tensor_add(si[:, cs], sf[:, cs], si[:, cs])
                        r = work.tile([P, GR], f32, tag="sg", name="r_t")
                        nc.vector.reciprocal_approx_fast(out=r[:], in_=si[:, cs])
                        nc.vector.tensor_mul(sf[:, cs], sf[:, cs], r[:])
                    for half in range(nhalf):
                        pst = pspool.tile([P, NT], f32, tag="ps", name="ps_t")
                        for k in range(KT):
                            nc.tensor.matmul(
                                pst[:], lhsT=wsl("h", k, i),
                                rhs=xsl(xcs[half], k),
                                start=(k == 0), stop=(k == KT - 1))
                        for c0 in range(0, NT, GR):
                            e0 = half * NT + c0
                            esl = slice(e0, e0 + GR)
                            psl = slice(c0, c0 + GR)
                            nc.scalar.activation(sg[:, esl], pst[:, psl],
                                                 AF.Sigmoid,
                                                 bias=bias_ap(2, i), scale=1.0)
                            nc.vector.scalar_tensor_tensor(
                                gt[:, esl], pst[:, psl], bias_ap(3, i),
                                sg[:, esl], op0=OP.add, op1=OP.max)
                            nc.vector.scalar_tensor_tensor(
                                gt[:, esl], sf[:, esl], 1.0, gt[:, esl],
                                op0=OP.subtract, op1=OP.mult)
                            hc = hpool.tile([P, GR], f32, tag="h",
                                            name=f"h{i}_t")
                            nc.vector.tensor_tensor_scan(
                                hc[:], sf[:, esl], gt[:, esl],
                                hprev[i][:, -1:],
                                op0=OP.mult, op1=OP.subtract)
                            hprev[i] = hc
                            nc.sync.dma_start(
                                out=out[i * P:(i + 1) * P,
                                        t0 + e0:t0 + e0 + GR],
                                in_=hc[:])

    nc.compile()
    return nc


def _in_maps(x, W_f, b_f, W_i, b_i, W_h, b_h):
    x = np.asarray(x, MM_NP)
    wT = {g: np.asarray(w, np.float32).T.astype(MM_NP)
          for g, w in (("f", W_f), ("i", W_i), ("h", W_h))}
    bs = {g: np.asarray(b, np.float32) for g, b in (("f", b_f), ("i", b_i), ("h", b_h))}

    maps = []
    for c in range(N_CORES):
        b, hh = divmod(c, 2)
        hsl = slice(hh * HSH, (hh + 1) * HSH)
        bias_pack = np.concatenate([
            bs["f"][hsl].reshape(IT, P).T,
            bs["i"][hsl].reshape(IT, P).T,
            bs["h"][hsl].reshape(IT, P).T,
            (bs["h"][hsl] + 0.5).reshape(IT, P).T,
        ], axis=1)
        # x pack: [p, c, kk, j, t] = xT[(2kk+j)*P + p, c*NT + t]
        xb = np.ascontiguousarray(x[b].T)                    # (DIN, T)
        xp = xb.reshape(KK, 2, P, NC, NT).transpose(2, 3, 0, 1, 4)
        # W pack: [p, kk, j, h] = W^T[(2kk+j)*P + p, h]
        wp = {g: wT[g][:, hsl].reshape(KK, 2, P, HSH).transpose(2, 0, 1, 3)
              for g in ("f", "i", "h")}
        maps.append({
            "xT": np.ascontiguousarray(xp.reshape(P, T * KT)),
            "wf": np.ascontiguousarray(wp["f"].reshape(P, KK * 2 * HSH)),
            "wi": np.ascontiguousarray(wp["i"].reshape(P, KK * 2 * HSH)),
            "wh": np.ascontiguousarray(wp["h"].reshape(P, KK * 2 * HSH)),
            "biases": np.ascontiguousarray(bias_pack, dtype=np.float32),
        })
    return maps


def kernel(x, W_f, b_f, W_i, b_i, W_h, b_h):
    global _COMPILED
    if _COMPILED is None:
        _COMPILED = _build()
    nc = _COMPILED

    res = run_bass_kernel_spmd(
        nc, _in_maps(x, W_f, b_f, W_i, b_i, W_h, b_h), list(range(N_CORES)))

    full = np.empty((B, T, DH), np.float32)
    for c in range(N_CORES):
        b, hh = divmod(c, 2)
        full[b, :, hh * HSH:(hh + 1) * HSH] = res.results[c]["out"].T
    return full
